# revision 49
# baseline (speedup 1.0000x reference)
"""Windowed multi-head self-attention Bass kernel for Trainium2.

Shapes (hardcoded): input [64, 256, 1536] fp32 (packed qkv, 32 heads x 16 dim),
rel_bias_table [127, 32] fp32. Output [64, 256, 512] fp32.

Sharding: data-parallel over the window axis B=64 across 8 NeuronCores
(8 windows per core). The tiny bias table is preprocessed on host into
per-head streaming blocks (bf16) and replicated to every core.

Per-core algorithm, processed in window PAIRS (wp = 2 windows):
  - Load input rows as [128, 2, 1536] fp32 SBUF tiles (one per window).
  - DVE casts the q/k sections to bf16; XBAR DMA-transposes produce
    qt/kt [c, cb, tok] bf16 tiles (no PE/PSUM involvement).
  - Batched SBUF->SBUF stitch DMAs scatter qt into QS_all (per
    16-partition channel group: 8 DMAs) and kt into KD_all (per
    64-partition half: 2 DMAs). QS_all/KD_all are double-buffered
    across pairs and carry static rows loaded once at startup:
      KD_all[hq]: kT of quad hq at partitions 64*(hq%2), one-hot U rows
        on the other half (m-chunk in free dim).
      QS_all[h]: qT_h at partitions 16*(h%8) (zeros elsewhere in that
        half), bias rows G'_h on the opposite half.
  - One K=128 bf16 matmul per (head, m-chunk) computes
    S^T = (q.k)^T + rel-pos bias straight into PSUM.
  - exp on ScalarE from PSUM in [128,1024] two-head batches -> bf16 P^T;
    1/sqrt(32) folded into the activation scale.
  - PV in bf16: lhsT = v' [128, 17] (v + ones column -> row 16 = softmax
    denominator, cast on GpSimd), rhs = P^T chunks, col-tiled 4 heads
    per PSUM tile.
  - PE-transpose back to [n, c] layout (f32r, 1.5 cyc/row), one
    reciprocal + one broadcast multiply per head-quad on VectorE,
    one contiguous output DMA per window.

The ScalarE exp stream (~133us) is the roofline for this kernel; all
other engines are kept below it and the schedule keeps ACT fed.
"""

import numpy as np
from contextlib import ExitStack

import concourse.bass as bass
import concourse.bacc as bacc
import concourse.tile as tile
from concourse import mybir
from concourse.bass_utils import run_bass_kernel_spmd

F32 = mybir.dt.float32
F32R = mybir.dt.float32r
BF16 = mybir.dt.bfloat16

NCORES = 8
B = 64
W = B // NCORES
N = 256
C = 1536
NH = 32
HD = 16
SCALE = float(NH) ** -0.5
NP = W // 2

# element pitches (in elements) of the mega tiles
QS_PITCH = NH * 2 * N          # [h 32][ws 2][n 256] bf16
KD_PITCH = 8 * 2 * 2 * 128     # [hq 8][ws 2][mch 2][m 128] bf16
QT_PITCH = 8 * 2 * N           # qkt: [b 8 (q:0-3, k:4-7)][ws 2][tok 256] bf16


def _build_kernel_body(ctx, tc, out, inp, gbias, uhot, ident, zeros):
    nc = tc.nc

    singles = ctx.enter_context(tc.tile_pool(name="singles", bufs=1))
    inpool = ctx.enter_context(tc.tile_pool(name="inpool", bufs=2))
    cpool = ctx.enter_context(tc.tile_pool(name="cpool", bufs=2))
    tpool = ctx.enter_context(tc.tile_pool(name="tpool", bufs=2))
    vpool = ctx.enter_context(tc.tile_pool(name="vpool", bufs=2))
    ppool = ctx.enter_context(tc.tile_pool(name="ppool", bufs=3))
    opool = ctx.enter_context(tc.tile_pool(name="opool", bufs=2))
    rpool = ctx.enter_context(tc.tile_pool(name="rpool", bufs=4))
    spool = ctx.enter_context(tc.tile_pool(name="spool", bufs=2))
    ps_s = ctx.enter_context(tc.tile_pool(name="ps_s", bufs=2, space="PSUM"))
    ps_pv = ctx.enter_context(tc.tile_pool(name="ps_pv", bufs=2, space="PSUM"))
    ps_to = ctx.enter_context(tc.tile_pool(name="ps_to", bufs=2, space="PSUM"))

    # --- static tiles ---
    id_t = singles.tile([128, 128], F32R, tag="ident")

    # QS_all[pp]: [128, 32 h, 2 ws, 256 n] bf16.
    # KD_all[pp]: [128, 8 hq, 2 ws, 2 mch, 128 m] bf16.
    QS = [singles.tile([128, NH, 2, N], BF16, tag=f"qs{p}", name=f"qs{p}")
          for p in range(2)]
    KD = [singles.tile([128, 8, 2, 2, 128], BF16, tag=f"kd{p}", name=f"kd{p}")
          for p in range(2)]

    def emit_zeros(p):
        # zero the q-half rows with memsets, split DVE/Pool (the q slots get
        # overwritten per pair; the 48 non-slot rows per head must stay 0)
        for cls in range(2):
            for cb in range(4):
                h0 = 8 * cb + 4 * cls
                dst = bass.AP(
                    tensor=QS[p].tensor,
                    offset=QS[p].offset + cls * 64 * QS_PITCH + h0 * 2 * N,
                    ap=[[QS_PITCH, 64], [2 * N, 4], [1, 2 * N]])
                eng = nc.vector if cb % 2 == 0 else nc.gpsimd
                eng.memset(dst, 0.0)

    def emit_statics(p, part=None):
        # bias rows (8 DMAs, ws-replicated in DRAM) + U rows (2 DMAs), all
        # on the SP HWDGE queue. part=0/1/2 emits a subset (for hook use).
        if part in (None, 0, 1):
            rng = range(2) if part is None else [part]
            for cls in rng:            # 0: h%8<4 (bias rows 64-127)
                for cb in range(4):
                    h0 = 8 * cb + 4 * cls
                    dst = bass.AP(
                        tensor=QS[p].tensor,
                        offset=(QS[p].offset + (1 - cls) * 64 * QS_PITCH
                                + h0 * 2 * N),
                        ap=[[QS_PITCH, 64], [2 * N, 4], [1, 2 * N]])
                    src = bass.AP(
                        tensor=gbias.tensor,
                        offset=gbias.offset + (4 * cls + cb) * 64 * 4 * 2 * N,
                        ap=[[4 * 2 * N, 64], [2 * N, 4], [1, 2 * N]])
                    nc.sync.dma_start(out=dst, in_=src)
        if part in (None, 2):
            for b in range(2):         # U rows on half b <- quads hq%2==1-b
                dst = bass.AP(
                    tensor=KD[p].tensor,
                    offset=KD[p].offset + 64 * b * KD_PITCH + (1 - b) * 512,
                    ap=[[KD_PITCH, 64], [1024, 4], [1, 512]])
                src = bass.AP(
                    tensor=uhot.tensor, offset=uhot.offset,
                    ap=[[512, 64], [0, 4], [1, 512]])
                nc.sync.dma_start(out=dst, in_=src)

    st = {}

    def emit_load(wp):
        xin = {}
        for ws in range(2):
            t = inpool.tile([128, 2, C], F32, tag=f"xin{ws}", name=f"xin{ws}")
            src = bass.AP(
                tensor=inp.tensor,
                offset=inp.offset + (2 * wp + ws) * N * C,
                ap=[[C, 128], [128 * C, 2], [1, C]])
            nc.sync.dma_start(out=t[:], in_=src)
            xin[ws] = t
        st[wp] = {"xin": xin}

    def emit_cast(wp, ws, ch):
        # q/k sections -> bf16. Pool in steady state (a stall there hurts
        # nothing); DVE for pair 0 where Pool is busy with static loads.
        xin = st[wp]["xin"]
        xb = st[wp].setdefault("xb", {})
        t = cpool.tile([128, 1024], BF16, tag=f"xb{ws}{ch}",
                       name=f"xb{ws}{ch}")
        eng = nc.vector if wp == 0 else nc.gpsimd
        eng.tensor_copy(t[:], xin[ws][:, ch, 0:1024])
        xb[(ws, ch)] = t

    def emit_vb(wp, ws, first):
        # v section -> bf16 [128, h, ch, 17] on GpSimd (two copies, one per
        # ch; ones column memset once per pool buffer)
        xin = st[wp]["xin"]
        vb = st[wp].setdefault("vb", {})
        t = vpool.tile([128, NH, 2, 17], BF16, tag=f"vb{ws}", name=f"vb{ws}")
        for ch in range(2):
            nc.gpsimd.tensor_copy(
                t[:, :, ch, 0:16],
                xin[ws][:, ch, 1024:1536].rearrange("p (h d) -> p h d", d=16))
        if first:
            nc.gpsimd.memset(t[:, :, :, 16:17], 1.0)
        vb[ws] = t

    def emit_xbar(wp, ws, ch):
        # XBAR transpose: xb [128 tok, 1024 c] -> qkt [128 c, b, tok]
        # (b 0-3 = q channel blocks, b 4-7 = k channel blocks)
        xb = st[wp]["xb"]
        if ws == 0 and ch == 0:
            st[wp]["qkt"] = tpool.tile([128, 8, 2, N], BF16, tag="qkt",
                                       name="qkt")
        dstt = st[wp]["qkt"]
        dst = bass.AP(
            tensor=dstt.tensor,
            offset=dstt.offset + ws * N + ch * 128,
            ap=[[QT_PITCH, 128], [2 * N, 8], [1, 128]])
        nc.sync.dma_start_transpose(dst, xb[(ws, ch)][:])

    def emit_stitch_q(wp, a):
        # qkt rows 16a:16a+16 (b 0-3) hold q^T of heads {a+8cb}; QS q slot
        # partitions are 16*(h%8) == 16a for every head.
        pp = wp % 2
        qkt = st[wp]["qkt"]
        dst = bass.AP(
            tensor=QS[pp].tensor,
            offset=QS[pp].offset + 16 * a * QS_PITCH + a * 2 * N,
            ap=[[QS_PITCH, 16], [8 * 2 * N, 4], [1, 2 * N]])
        src = bass.AP(
            tensor=qkt.tensor,
            offset=qkt.offset + 16 * a * QT_PITCH,
            ap=[[QT_PITCH, 16], [2 * N, 4], [1, 2 * N]])
        nc.sync.dma_start(out=dst, in_=src)

    def emit_stitch_k(wp, b):
        # qkt partition half b (b-blocks 4-7) holds kT of quads {2cb+b}
        pp = wp % 2
        qkt = st[wp]["qkt"]
        dst = bass.AP(
            tensor=KD[pp].tensor,
            offset=KD[pp].offset + 64 * b * KD_PITCH + b * 512,
            ap=[[KD_PITCH, 64], [1024, 4], [1, 512]])
        src = bass.AP(
            tensor=qkt.tensor,
            offset=qkt.offset + 64 * b * QT_PITCH + 4 * 2 * N,
            ap=[[QT_PITCH, 64], [2 * N, 4], [1, 512]])
        nc.sync.dma_start(out=dst, in_=src)

    pending_out = []
    pending_pv = []

    def flush_out():
        while pending_out:
            w0, oacc0 = pending_out.pop(0)
            dst = bass.AP(
                tensor=out.tensor,
                offset=out.offset + w0 * N * 512,
                ap=[[512, 128], [128 * 512, 2], [1, 512]])
            nc.sync.dma_start(out=dst, in_=oacc0[:])

    def flush_pv():
        # PV + normalization tail for the previous head-group; deferred one
        # hg so the PE stream never waits on the exp results it just queued
        while pending_pv:
            hg, ppair, vb, oacc = pending_pv.pop(0)
            pv = ps_pv.tile([128, 256], F32, tag="pv", name="pv")
            for j in range(4):
                sub, par = divmod(j, 2)
                h = 4 * hg + j
                pt = ppair[sub]
                for mch in range(2):
                    qq = 2 * mch + par
                    nc.tensor.matmul(
                        pv[32 * j:32 * j + 17, :],
                        lhsT=vb[:, h, mch, :],
                        rhs=pt[:, qq * 256:(qq + 1) * 256],
                        start=(mch == 0),
                        stop=(mch == 1),
                        tile_position=(0, 32 * j),
                    )

            pvs = spool.tile([128, 256], F32R, tag="pvs", name="pvs")
            nc.vector.tensor_copy(pvs[:], pv[:])
            tro = ps_to.tile([128, 2, 128], F32R, tag="tro", name="tro")
            for ch in range(2):
                nc.tensor.transpose(tro[:, ch, :],
                                    pvs[:, ch * 128:(ch + 1) * 128],
                                    id_t[:])
            trv = tro.bitcast(F32).rearrange("p c (j x) -> p c j x", x=32)
            rcp = rpool.tile([128, 2, 4, 1], F32, tag="rcp", name="rcp")
            nc.vector.reciprocal(rcp[:], trv[:, :, :, 16:17])
            rb = rcp[:]
            rbcast = bass.AP(
                tensor=rb.tensor, offset=rb.offset,
                ap=[rb.ap[0], rb.ap[1], rb.ap[2], [0, 16]])
            nc.vector.tensor_mul(
                oacc[:, :, 64 * hg:64 * hg + 64].rearrange(
                    "p c (j d) -> p c j d", d=16),
                trv[:, :, :, 0:16],
                rbcast,
            )

    def emit_compute(wp, ws, hooks=()):
        # hooks: list of (hg_index, fn) to interleave pipeline work
        pp = wp % 2
        vb = st[wp]["vb"][ws]
        w = 2 * wp + ws
        oacc = opool.tile([128, 2, 512], F32, tag="oacc", name="oacc")
        hookmap = {}
        for hgi, fn in hooks:
            hookmap.setdefault(hgi, []).append(fn)

        for hg in range(8):
            ppair = []
            for sub in range(2):
                # ps free layout: [mch 2, par 2, n 256]; one matmul covers
                # both par heads (rhs [128, 2, 256], shared lhsT weights).
                ps = ps_s.tile([128, 1024], F32, tag="scores", name="scores")
                h = 4 * hg + 2 * sub
                for mch in range(2):
                    nc.tensor.matmul(
                        ps[:, mch * 512:(mch + 1) * 512],
                        lhsT=KD[pp][:, hg, ws, mch, :],
                        rhs=QS[pp][:, h:h + 2, ws, :],
                        start=True,
                        stop=True,
                    )
                pt = ppool.tile([128, 1024], BF16, tag="pt", name="pt")
                nc.scalar.activation(
                    pt[:], ps[:], mybir.ActivationFunctionType.Exp,
                    scale=SCALE,
                )
                ppair.append(pt)

            flush_pv()
            pending_pv.append((hg, ppair, vb, oacc))
            if hg == 2:
                # flush the previous window's output DMA here: its last
                # multiply has long finished, so SP never blocks on it
                flush_out()
            for fn in hookmap.get(hg, ()):
                fn()

        pending_out.append((w, oacc))

    # ---- software-pipelined pair loop ----
    def prelude(wp, skip_load=False):
        if not skip_load:
            emit_load(wp)
        for ws in range(2):
            for ch in range(2):
                emit_cast(wp, ws, ch)
        for ws in range(2):
            emit_vb(wp, ws, first=(wp < 2))
        for ws in range(2):
            for ch in range(2):
                emit_xbar(wp, ws, ch)
        for a in range(8):
            emit_stitch_q(wp, a)
        for b in range(2):
            emit_stitch_k(wp, b)

    emit_load(0)
    nc.sync.dma_start(out=id_t[:], in_=ident)
    emit_zeros(0)
    emit_statics(0)
    prelude(0, skip_load=True)
    emit_zeros(1)
    for wp in range(NP):
        nxt = wp + 1
        if nxt < NP:
            # spread next-pair pipeline work across this pair's compute
            hooks0 = [
                (0, lambda n=nxt: emit_load(n)),
                (1, lambda n=nxt: emit_cast(n, 0, 0)),
                (2, lambda n=nxt: emit_cast(n, 0, 1)),
                (3, lambda n=nxt: emit_cast(n, 1, 0)),
                (4, lambda n=nxt: emit_cast(n, 1, 1)),
                (5, lambda n=nxt: [emit_vb(n, 0, n < 2),
                                   emit_xbar(n, 0, 0)]),
                (6, lambda n=nxt: [emit_vb(n, 1, n < 2),
                                   emit_xbar(n, 0, 1)]),
                (7, lambda n=nxt: emit_xbar(n, 1, 0)),
            ]
            if wp == 0:
                # pp1 statics stream behind load(1) on the SP queue
                hooks0 += [
                    (1, lambda: emit_statics(1, 0)),
                    (2, lambda: emit_statics(1, 1)),
                    (3, lambda: emit_statics(1, 2)),
                ]

            hooks1 = [
                (0, lambda n=nxt: emit_xbar(n, 1, 1)),
                (1, lambda n=nxt: [emit_stitch_q(n, 0),
                                   emit_stitch_q(n, 1)]),
                (2, lambda n=nxt: [emit_stitch_q(n, 2),
                                   emit_stitch_q(n, 3)]),
                (3, lambda n=nxt: [emit_stitch_q(n, 4),
                                   emit_stitch_q(n, 5)]),
                (4, lambda n=nxt: [emit_stitch_q(n, 6),
                                   emit_stitch_q(n, 7)]),
                (5, lambda n=nxt: [emit_stitch_k(n, 0),
                                   emit_stitch_k(n, 1)]),
            ]
        else:
            hooks0 = hooks1 = []
        emit_compute(wp, 0, hooks0)
        emit_compute(wp, 1, hooks1)
        st.pop(wp - 1, None)
    flush_pv()
    flush_out()


def build_nc():
    nc = bacc.Bacc(
        "TRN2", target_bir_lowering=False, debug=False, num_devices=NCORES
    )
    inp = nc.dram_tensor("inp", [W, N, C], F32, kind="ExternalInput").ap()
    # gbias: [8 (cls,cb), 64 p, 4 a, 512 (ws,n)] bf16 (bias-half rows only,
    # ws-replicated so one DMA covers a head's full (ws, n) extent)
    gbias = nc.dram_tensor("gbias", [8, 64, 4, 2 * N], BF16,
                           kind="ExternalInput").ap()
    # uhot: [64 i, 512 (ws,mch,m)] bf16, ws-replicated
    uhot = nc.dram_tensor("uhot", [64, 512], BF16, kind="ExternalInput").ap()
    ident = nc.dram_tensor("ident", [128, 128], F32R,
                           kind="ExternalInput").ap()
    zeros = nc.dram_tensor("zeros", [2 * N], BF16, kind="ExternalInput").ap()
    out = nc.dram_tensor("out", [W, N, NH * HD], F32,
                         kind="ExternalOutput").ap()
    with tile.TileContext(nc) as tc:
        with ExitStack() as ctx:
            _build_kernel_body(ctx, tc, out, inp, gbias, uhot, ident, zeros)
    nc.compile()
    return nc


def _to_bf16(x):
    import ml_dtypes
    return np.asarray(x, np.float32).astype(ml_dtypes.bfloat16)


def _host_consts(table):
    # G'[h, i, n] = table[n//4 - i + 63, h] * sqrt(32), i in [0, 64)
    j = np.arange(N) // 4
    i0 = np.arange(64)
    idx = j[None, :] - i0[:, None] + 63  # [64, 256]
    g = table[idx]  # [64, 256, NH]
    gfull = np.ascontiguousarray(np.transpose(g, (2, 0, 1))) * np.float32(
        1.0 / SCALE)  # [NH, 64, 256]
    # gbias layout: [8 (cls*4+cb), 64 p, 4 a, 512 (ws-replicated n)]
    gb = np.zeros((8, 64, 4, 2 * N), np.float32)
    for cls in range(2):
        for cb in range(4):
            for a in range(4):
                h = 8 * cb + 4 * cls + a
                gb[cls * 4 + cb, :, a, 0:N] = gfull[h]
                gb[cls * 4 + cb, :, a, N:] = gfull[h]
    # U[i, mch, m] = 1 if (mch*128 + m)//4 == i ; ws-replicated to [64, 512]
    mg = np.arange(N) // 4
    u = (mg[None, :] == np.arange(64)[:, None]).astype(np.float32)  # [64,256]
    u = np.concatenate([u, u], axis=1)  # [64, 512] = (ws, mch*m)
    ident = np.eye(128, dtype=np.float32)
    zeros = np.zeros((2 * N,), np.float32)
    return (_to_bf16(gb), _to_bf16(u), ident, _to_bf16(zeros))


_NC_CACHE = None


def kernel(input, rel_bias_table):
    global _NC_CACHE
    x = np.ascontiguousarray(np.asarray(input, dtype=np.float32))
    tbl = np.asarray(rel_bias_table, dtype=np.float32)
    assert x.shape == (B, N, C), x.shape
    assert tbl.shape == (127, NH), tbl.shape

    if _NC_CACHE is None:
        _NC_CACHE = build_nc()
    nc = _NC_CACHE

    gbias, uhot, ident, zeros = _host_consts(tbl)
    in_maps = [
        {
            "inp": np.ascontiguousarray(x[i * W:(i + 1) * W]),
            "gbias": gbias,
            "uhot": uhot,
            "ident": ident,
            "zeros": zeros,
        }
        for i in range(NCORES)
    ]
    res = run_bass_kernel_spmd(nc, in_maps, list(range(NCORES)))
    return np.concatenate([res.results[i]["out"] for i in range(NCORES)],
                          axis=0)
